# revision 1
# baseline (speedup 1.0000x reference)
"""Trainium2 Bass kernel for nn_MedicalVisionTransformer (MoE-LoRA ViT).

Strategy: data-parallel over batch (8 cores x 8 batch items). Each core holds
its 256-token (8 batches x 32 slots: 30 real + 2 pad) residual stream in SBUF
for all 12 layers; only weights stream from HBM in fp16. MoE LoRA experts are
collapsed algebraically (rank-8 C matrices; per-expert LayerNorm folded into
per-token scalars via B_down Gram matrices) so no [B,S,E,H]/[B,S,E,D] tensor
is ever materialized.
"""

import sys

sys.path.insert(0, "/opt/trn_rl_repo")

import numpy as np

import concourse.bass as bass
import concourse.mybir as mybir
import concourse.tile as tile
from concourse import bacc
from concourse import bass_utils

f32 = np.float32
F32 = mybir.dt.float32
F16 = mybir.dt.float16
F32R = mybir.dt.float32r

B, SR, D, H, L, NH, ND, E, RK = 64, 29, 768, 3072, 12, 12, 14, 15, 8
S = SR + 1
SCALE = f32(16.0 / 8.0)
NE = L // 2
DH = D // NH
NC = 8
BPC = B // NC          # batches per core
TS = 32                # token slot per batch (30 real + 2 pad)
NT = BPC * TS          # 256 tokens per core
D2 = D // 2            # 384
ER = E * RK            # 120
KC = D // 128          # 6 feature chunks
HC = H // 128          # 24 hidden chunks

AluOp = mybir.AluOpType
Act = mybir.ActivationFunctionType


# ----------------------------------------------------------------------------
# Host-side weight preparation (pure numpy; done once per kernel() call)
# ----------------------------------------------------------------------------

def _prep(inputs):
    P = {}
    qs = f32(1.0 / np.sqrt(DH))

    Wqk = np.zeros((L, 3, KC, 128, 4 * 128), np.float16)
    bqk = np.zeros((L, 128, 12), f32)
    Wv = np.zeros((L, KC, 128, D), np.float16)
    bvr = np.zeros((L, 1, D), np.float16)
    Wao = np.zeros((L, KC, 128, D), np.float16)
    baor = np.zeros((L, 1, D), np.float16)
    Wi = np.zeros((L, 6, KC, 128, 4 * 128), np.float16)
    bi = np.zeros((L, 128, HC), f32)
    Wo = np.zeros((L, HC, 128, D), np.float16)
    bor = np.zeros((L, 1, D), np.float16)

    for i in range(L):
        g1, b1 = inputs['ln1_g'][i], inputs['ln1_b'][i]
        g2, b2 = inputs['ln2_g'][i], inputs['ln2_b'][i]
        WqT = (inputs['Wq'][i] * g1[None, :]).T * qs      # [in, out]
        WkT = (inputs['Wk'][i] * g1[None, :]).T
        bq = (b1 @ inputs['Wq'][i].T + inputs['bq'][i]) * qs
        bk = b1 @ inputs['Wk'][i].T + inputs['bk'][i]
        qk = np.concatenate([WqT, WkT], axis=1)           # [768, 1536]
        for g in range(3):
            for c in range(KC):
                Wqk[i, g, c] = qk[c * 128:(c + 1) * 128,
                                  g * 512:(g + 1) * 512].astype(np.float16)
        bqkv = np.concatenate([bq, bk])                   # [1536]
        for fc in range(12):
            bqk[i, :, fc] = bqkv[fc * 128:(fc + 1) * 128]
        WvT = (inputs['Wv'][i] * g1[None, :]).T
        for c in range(KC):
            Wv[i, c] = WvT[c * 128:(c + 1) * 128].astype(np.float16)
        bvr[i, 0] = (b1 @ inputs['Wv'][i].T + inputs['bv'][i]).astype(np.float16)
        WaoT = inputs['Wao'][i].T
        for c in range(KC):
            Wao[i, c] = WaoT[c * 128:(c + 1) * 128].astype(np.float16)
        baor[i, 0] = inputs['bao'][i].astype(np.float16)
        WiT = (inputs['Wi'][i] * g2[None, :]).T           # [768, 3072]
        for g in range(6):
            for c in range(KC):
                Wi[i, g, c] = WiT[c * 128:(c + 1) * 128,
                                  g * 512:(g + 1) * 512].astype(np.float16)
        bih = (b2 @ inputs['Wi'][i].T + inputs['bi'][i])  # [3072]
        for hc in range(HC):
            bi[i, :, hc] = bih[hc * 128:(hc + 1) * 128]
        WoT = inputs['Wo'][i].T                            # [3072, 768]
        for c in range(HC):
            Wo[i, c] = WoT[c * 128:(c + 1) * 128].astype(np.float16)
        bor[i, 0] = inputs['bo'][i].astype(np.float16)

    P.update(Wqk=Wqk, bqk=bqk, Wv=Wv, bvr=bvr, Wao=Wao, baor=baor,
             Wi=Wi, bi=bi, Wo=Wo, bor=bor)

    # MoE / classifier tensors
    AuT = np.zeros((NE, 128, KC, ER), np.float16)
    lupbr = np.zeros((NE, 1, ER), np.float16)
    AdT = np.zeros((NE, 128, HC, ER), np.float16)
    Cblk = np.zeros((NE, ER, ER), np.float16)
    cW1 = np.zeros((NE, ND, KC, 128, D2), np.float16)
    cb1r = np.zeros((NE, 1, ND * D2), np.float16)
    clngB = np.zeros((NE, 8, ND, D2), np.float16)
    clnbB = np.zeros((NE, 8, ND, D2), np.float16)
    cw2rep = np.zeros((NE, 8, ND, D2), np.float16)
    cb2rep = np.zeros((NE, 8, ND), f32)
    BdfD = np.zeros((NE, 128, KC, ER), np.float16)
    Bdfr = np.zeros((NE, ER, D), np.float16)
    BdmSeg1 = np.zeros((NE, ER, E), np.float16)
    GramB = np.zeros((NE, ER, ER), np.float16)

    for e in range(NE):
        i = 2 * e
        g2, b2 = inputs['ln2_g'][i], inputs['ln2_b'][i]
        Au = inputs['A_up'][e]; Bu = inputs['B_up'][e]
        Ad = inputs['A_down'][e]; Bd = inputs['B_down'][e]
        AuTf = np.concatenate([(Au[ee] * g2[None, :]).T for ee in range(E)], axis=1)
        for c in range(KC):
            AuT[e, :, c, :] = AuTf[c * 128:(c + 1) * 128].astype(np.float16)
        lupbr[e, 0] = np.concatenate([b2 @ Au[ee].T for ee in range(E)]).astype(np.float16)
        AdTf = np.concatenate([Ad[ee].T for ee in range(E)], axis=1)   # [H, 120]
        for c in range(HC):
            AdT[e, :, c, :] = AdTf[c * 128:(c + 1) * 128].astype(np.float16)
        for ee in range(E):
            Cm = Ad[ee] @ Bu[ee]                                        # [r, r']
            Cblk[e, ee * RK:(ee + 1) * RK, ee * RK:(ee + 1) * RK] = \
                (SCALE * Cm.T).astype(np.float16)
        Bdf = np.concatenate([Bd[ee].T for ee in range(E)], axis=0) * SCALE  # [120, D]
        Bdfr[e] = Bdf.astype(np.float16)
        BdfDf = (2.0 * Bdf.T / f32(D))                                  # [D, 120] (2x for cross term)
        for c in range(KC):
            BdfD[e, :, c, :] = BdfDf[c * 128:(c + 1) * 128].astype(np.float16)
        Bdm = Bdf.mean(axis=1)                                          # [120]
        for ee in range(E):
            BdmSeg1[e, ee * RK:(ee + 1) * RK, ee] = Bdm[ee * RK:(ee + 1) * RK].astype(np.float16)
            sl = slice(ee * RK, (ee + 1) * RK)
            GramB[e][sl, sl] = (Bdf[sl] @ Bdf[sl].T) / f32(D)
        # classifier; z rows ordered (d, tg, bl)
        cW1e = inputs['cW1'][e]
        for d in range(ND):
            W1T = cW1e[d].T                                             # [768, 384]
            for c in range(KC):
                cW1[e, d, c] = W1T[c * 128:(c + 1) * 128].astype(np.float16)
        cb1r[e, 0] = inputs['cb1'][e].astype(np.float16).ravel()
        for d in range(ND):
            for j in range(8):     # j = tg*4 + bl
                clngB[e, j, d] = inputs['clng'][e][d]
                clnbB[e, j, d] = inputs['clnb'][e][d]
                cw2rep[e, j, d] = inputs['cW2'][e][d]
                cb2rep[e, j, d] = inputs['cb2'][e][d]

    P.update(AuT=AuT, lupbr=lupbr, AdT=AdT, Cblk=Cblk, cW1=cW1, cb1r=cb1r,
             clngB=clngB, clnbB=clnbB, cw2rep=cw2rep, cb2rep=cb2rep,
             BdfD=BdfD, Bdfr=Bdfr, BdmSeg1=BdmSeg1, GramB=GramB)

    # e-independent constants
    SegSel0 = np.zeros((ER, E), np.float16)
    for ee in range(E):
        SegSel0[ee * RK:(ee + 1) * RK, ee] = 1.0
    RepSel = np.zeros((E, ER), np.float16)
    for ee in range(E):
        RepSel[ee, ee * RK:(ee + 1) * RK] = 1.0
    P.update(SegSel0=SegSel0, RepSel=RepSel)

    mask = inputs['mask']; cnt = mask.sum(axis=0)
    # pooling: per token group, cols (d, bl)
    Mpool = np.zeros((2, 128, 4 * ND), np.float16)
    for tg in range(2):
        for d in range(ND):
            for bl in range(4):
                col = d * 4 + bl
                Mpool[tg, bl * TS + 1: bl * TS + 1 + SR, col] = \
                    (mask[:, d] / cnt[d]).astype(np.float16)
    P['Mpool'] = Mpool

    # attention mask (rank-5 additive init): rows [ones; Bq(4)] x [−30000; 30000*Bk]
    attL = np.zeros((2, 5, 128), np.float16)
    attR = np.zeros((2, 5, NT), np.float16)
    for tg in range(2):
        attL[tg, 0, :] = 1.0
        attR[tg, 0, :] = -30000.0
        for j in range(4):
            b = tg * 4 + j
            attL[tg, 1 + j, j * TS:(j + 1) * TS] = 1.0
            ks = np.arange(NT)
            ok = (ks // TS == b) & (ks % TS < S)
            attR[tg, 1 + j, ok] = 30000.0
    P.update(attL=attL, attR=attR)

    maskS = np.zeros((TS, ND), f32)
    maskS[1:1 + SR] = mask
    P['maskS'] = np.tile(maskS, (4, 1))          # [128, ND]

    Ind2 = np.zeros((2, 8, 128), np.float16)
    for tg in range(2):
        for bl in range(4):
            j = tg * 4 + bl
            Ind2[tg, j, bl * TS:(bl + 1) * TS] = 1.0
    P.update(Ind2=Ind2)

    P['fgB'] = np.tile(inputs['fg'][None, :], (128, 1)).astype(f32)
    P['fbB'] = np.tile(inputs['fb'][None, :], (128, 1)).astype(f32)
    return P


def _shard_x0(inputs):
    """Per-core [2, 128, 768] initial residual streams."""
    cls = np.asarray(inputs['cls_token'][0, 0], f32)
    rf = np.asarray(inputs['region_features'], f32)
    shards = []
    for c in range(NC):
        x0 = np.zeros((NT, D), f32)
        for bl in range(BPC):
            b = c * BPC + bl
            x0[bl * TS] = cls
            x0[bl * TS + 1: bl * TS + 1 + SR] = rf[b]
        shards.append(x0.reshape(2, 128, D))
    return shards


# ----------------------------------------------------------------------------
# Bass/Tile program
# ----------------------------------------------------------------------------

def _build(sim_gelu=False):
    nc = bacc.Bacc("TRN2", target_bir_lowering=False, debug=False)

    def din(name, shape, dt):
        return nc.dram_tensor(name, list(shape), dt, kind="ExternalInput")

    t_x0 = din("x0", (2, 128, D), F32)
    t_Wqk = din("Wqk", (L, 3, KC, 128, 4 * 128), F16)
    t_bqk = din("bqk", (L, 128, 12), F32)
    t_Wv = din("Wv", (L, KC, 128, D), F16)
    t_bvr = din("bvr", (L, 1, D), F16)
    t_Wao = din("Wao", (L, KC, 128, D), F16)
    t_baor = din("baor", (L, 1, D), F16)
    t_Wi = din("Wi", (L, 6, KC, 128, 4 * 128), F16)
    t_bi = din("bi", (L, 128, HC), F32)
    t_Wo = din("Wo", (L, HC, 128, D), F16)
    t_bor = din("bor", (L, 1, D), F16)
    t_AuT = din("AuT", (NE, 128, KC, ER), F16)
    t_lupbr = din("lupbr", (NE, 1, ER), F16)
    t_AdT = din("AdT", (NE, 128, HC, ER), F16)
    t_Cblk = din("Cblk", (NE, ER, ER), F16)
    t_cW1 = din("cW1", (NE, ND, KC, 128, D2), F16)
    t_cb1r = din("cb1r", (NE, 1, ND * D2), F16)
    t_clngB = din("clngB", (NE, 8, ND, D2), F16)
    t_clnbB = din("clnbB", (NE, 8, ND, D2), F16)
    t_cw2rep = din("cw2rep", (NE, 8, ND, D2), F16)
    t_cb2rep = din("cb2rep", (NE, 8, ND), F32)
    t_BdfD = din("BdfD", (NE, 128, KC, ER), F16)
    t_Bdfr = din("Bdfr", (NE, ER, D), F16)
    t_BdmSeg1 = din("BdmSeg1", (NE, ER, E), F16)
    t_GramB = din("GramB", (NE, ER, ER), F16)
    t_SegSel0 = din("SegSel0", (ER, E), F16)
    t_RepSel = din("RepSel", (E, ER), F16)
    t_Mpool = din("Mpool", (2, 128, 4 * ND), F16)
    t_attL = din("attL", (2, 5, 128), F16)
    t_attR = din("attR", (2, 5, NT), F16)
    t_maskS = din("maskS", (128, ND), F32)
    t_Ind2 = din("Ind2", (2, 8, 128), F16)
    t_fgB = din("fgB", (128, D), F32)
    t_fbB = din("fbB", (128, D), F32)
    t_out = nc.dram_tensor("out", [2, 128, D], F32, kind="ExternalOutput")

    with tile.TileContext(nc) as tc:
        with (
            tc.tile_pool(name="const", bufs=1) as cpool,
            tc.tile_pool(name="resid", bufs=1) as hpool,
            tc.tile_pool(name="wstream", bufs=4) as wpool,
            tc.tile_pool(name="wbias", bufs=2) as bpool,
            tc.tile_pool(name="wmoe", bufs=1) as wmpool,
            tc.tile_pool(name="acts", bufs=1) as apool,
            tc.tile_pool(name="scrA", bufs=3) as sapool,
            tc.tile_pool(name="scrB", bufs=1) as spool,
            tc.tile_pool(name="small", bufs=2) as mpool,
            tc.tile_pool(name="psX", bufs=4, space="PSUM") as psX,
            tc.tile_pool(name="psY", bufs=4, space="PSUM") as psY,
        ):
            dma = nc.sync.dma_start

            def act_gelu(dst, src, bias=None):
                if not sim_gelu:
                    if bias is None:
                        nc.scalar.activation(dst, src, Act.Gelu)
                    else:
                        nc.scalar.activation(dst, src, Act.Gelu, bias=bias,
                                             scale=1.0)
                    return
                shp = list(dst.shape)
                y = sapool.tile(shp, F32, tag="gel_y", name="gel_y")
                if bias is None:
                    nc.scalar.activation(y[:], src, Act.Identity)
                else:
                    nc.scalar.activation(y[:], src, Act.Identity, bias=bias,
                                         scale=1.0)
                u = sapool.tile(shp, F32, tag="gel_u", name="gel_u")
                nc.vector.tensor_tensor(out=u[:], in0=y[:], in1=y[:],
                                        op=AluOp.mult)
                nc.vector.tensor_tensor(out=u[:], in0=u[:], in1=y[:],
                                        op=AluOp.mult)
                nc.vector.tensor_scalar(out=u[:], in0=u[:], scalar1=0.044715,
                                        scalar2=None, op0=AluOp.mult)
                nc.vector.tensor_tensor(out=u[:], in0=u[:], in1=y[:],
                                        op=AluOp.add)
                nc.scalar.activation(u[:], u[:], Act.Tanh, scale=0.7978845608)
                nc.vector.tensor_scalar(out=u[:], in0=u[:], scalar1=1.0,
                                        scalar2=0.5, op0=AluOp.add,
                                        op1=AluOp.mult)
                nc.vector.tensor_tensor(out=dst, in0=u[:], in1=y[:],
                                        op=AluOp.mult)

            # ---------------- constants ----------------
            ident = cpool.tile([128, 128], F16)
            from concourse.masks import make_identity
            make_identity(nc, ident[:])
            ident32 = cpool.tile([128, 128], F32)
            make_identity(nc, ident32[:])
            onesc = cpool.tile([1, 512], F16)   # K=1 matmul lhsT/rhs ones
            nc.vector.memset(onesc[:], 1.0)
            ones15 = cpool.tile([E, 1], F16)
            nc.vector.memset(ones15[:], 1.0)
            c_attL = cpool.tile([5, 2, 128], F16)
            c_attR = cpool.tile([5, 2, NT], F16)
            for tg in range(2):
                dma(c_attL[:, tg, :], t_attL[tg, :, :])
                dma(c_attR[:, tg, :], t_attR[tg, :, :])
            c_Mpool = cpool.tile([128, 2, 4 * ND], F16)
            for tg in range(2):
                dma(c_Mpool[:, tg, :], t_Mpool[tg, :, :])
            c_maskS = cpool.tile([128, ND], F32); dma(c_maskS[:], t_maskS[:])
            c_Ind2 = cpool.tile([8, 2, 128], F16)
            for tg in range(2):
                dma(c_Ind2[:, tg, :], t_Ind2[tg, :, :])
            c_SegSel0 = cpool.tile([ER, E], F16); dma(c_SegSel0[:], t_SegSel0[:])
            c_RepSel = cpool.tile([E, ER], F16); dma(c_RepSel[:], t_RepSel[:])
            c_fgB = cpool.tile([128, D], F32); dma(c_fgB[:], t_fgB[:])
            c_fbB = cpool.tile([128, D], F32); dma(c_fbB[:], t_fbB[:])
            eps12 = cpool.tile([128, 1], F32); nc.vector.memset(eps12[:], 1e-12)
            eps5a = cpool.tile([128, 1], F32); nc.vector.memset(eps5a[:], 1e-5)

            # ---------------- persistent activations ----------------
            h = hpool.tile([128, 2, D], F32)
            res1 = hpool.tile([128, 2, D], F32)
            for tg in range(2):
                dma(h[:, tg, :], t_x0[tg, :, :])

            def layernorm_16(src, dst, eps_tile):
                """src [128, 2, D] f32 -> dst [128, 2, D] f16, plain LN rows."""
                for tg in range(2):
                    st = mpool.tile([128, 3, 6], F32, tag="lnst")
                    xs = src[:, tg, :].rearrange("p (a b) -> p a b", a=3)
                    for a in range(3):
                        nc.vector.bn_stats(st[:, a, :], xs[:, a, :])
                    mv = mpool.tile([128, 2], F32, tag="lnmv")
                    nc.vector.bn_aggr(mv[:], st[:])
                    sq = mpool.tile([128, 1], F32, tag="lnsq")
                    nc.scalar.activation(sq[:], mv[:, 1:2], Act.Sqrt,
                                         bias=eps_tile[:], scale=1.0)
                    rst = mpool.tile([128, 1], F32, tag="lnrs")
                    nc.vector.reciprocal(rst[:], sq[:])
                    nc.vector.tensor_scalar(
                        out=dst[:, tg, :], in0=src[:, tg, :],
                        scalar1=mv[:, 0:1], scalar2=rst[:],
                        op0=AluOp.subtract, op1=AluOp.mult)

            def transpose6(src32, dst):
                """src32 [128, 2, D] f32 -> dst [128, KC, 256] f16 (feature-major)."""
                for c in range(KC):
                    for tg in range(2):
                        pt = psY.tile([128, 128], F32, tag="sm")
                        nc.tensor.transpose(pt[:], src32[:, tg, c * 128:(c + 1) * 128],
                                            ident32[:])
                        nc.vector.tensor_copy(dst[:, c, tg * 128:(tg + 1) * 128], pt[:])

            # ---------------- layers ----------------
            for i in range(L):
                even = (i % 2 == 0)
                e = i // 2

                # LN1 -> n1 (f16) -> n1T
                n1 = apool.tile([128, 2, D], F32, tag="n1")
                layernorm_16(h, n1, eps12)
                n1T = apool.tile([128, KC, 256], F16, tag="n1T")
                transpose6(n1, n1T)

                # QK^T (transposed out, fused per-partition bias)
                b_qk = bpool.tile([128, 12], F32, tag="bqk")
                dma(b_qk[:], t_bqk[i, :, :])
                qkT = apool.tile([128, 12, 256], F16, tag="bigact")
                for g in range(3):
                    pss = [psX.tile([128, 256], F32, tag="acc", name=f"qk{g}{j}")
                           for j in range(4)]
                    for c in range(KC):
                        w_ch = wpool.tile([128, 4 * 128], F16, tag="wchunk")
                        dma(w_ch[:], t_Wqk[i, g, c, :, :])
                        for j in range(4):
                            nc.tensor.matmul(pss[j][:],
                                             w_ch[:, j * 128:(j + 1) * 128],
                                             n1T[:, c, :],
                                             start=(c == 0), stop=(c == KC - 1))
                    for j in range(4):
                        fc = g * 4 + j
                        nc.vector.tensor_scalar(out=qkT[:, fc, :], in0=pss[j][:],
                                                scalar1=b_qk[:, fc:fc + 1],
                                                scalar2=None, op0=AluOp.add)

                # V (untransposed: [tok, dv])
                b_v = bpool.tile([1, D], F16, tag="bv")
                dma(b_v[:], t_bvr[i, :, :])
                V = apool.tile([128, 2, D], F16, tag="V")
                psv = [[psX.tile([128, 384], F32, tag="acc", name=f"v{tg}{n}")
                        for n in range(2)] for tg in range(2)]
                for tg in range(2):
                    for n in range(2):
                        sl = slice(n * 384, (n + 1) * 384)
                        nc.tensor.matmul(psv[tg][n][:], onesc[:1, :128],
                                         b_v[:, sl], start=True, stop=False)
                for c in range(KC):
                    w_ch = wpool.tile([128, D], F16, tag="wchunk", name="wv_c")
                    dma(w_ch[:], t_Wv[i, c, :, :])
                    for tg in range(2):
                        for n in range(2):
                            sl = slice(n * 384, (n + 1) * 384)
                            nc.tensor.matmul(psv[tg][n][:],
                                             n1T[:, c, tg * 128:(tg + 1) * 128],
                                             w_ch[:, sl],
                                             start=False, stop=(c == KC - 1))
                for tg in range(2):
                    for n in range(2):
                        sl = slice(n * 384, (n + 1) * 384)
                        nc.vector.tensor_copy(V[:, tg, sl], psv[tg][n][:])

                # attention
                oT = apool.tile([128, KC, 256], F16, tag="oT")
                for tg in range(2):
                    for dvc in range(KC):
                        pso = psY.tile([128, 128], F32, tag="sm")
                        for half in range(2):
                            hh = dvc * 2 + half
                            fc, off = hh // 2, (hh % 2) * 64
                            ps = psY.tile([128, 256], F32, tag="sm")
                            nc.tensor.matmul(ps[:], c_attL[:, tg, :], c_attR[:, tg, :],
                                             start=True, stop=False)
                            nc.tensor.matmul(
                                ps[:],
                                qkT[off:off + 64, fc, tg * 128:(tg + 1) * 128],
                                qkT[off:off + 64, 6 + fc, :],
                                start=False, stop=True)
                            ex = sapool.tile([128, 256], F32, tag="ex")
                            rs = mpool.tile([128, 1], F32, tag="rs")
                            nc.scalar.activation(ex[:], ps[:], Act.Exp,
                                                 accum_out=rs[:])
                            rn = mpool.tile([128, 1], F32, tag="rn")
                            nc.vector.reciprocal(rn[:], rs[:])
                            att = sapool.tile([128, 256], F32, tag="att")
                            nc.vector.tensor_scalar(out=att[:], in0=ex[:],
                                                    scalar1=rn[:], scalar2=None,
                                                    op0=AluOp.mult)
                            attT = sapool.tile([128, 2, 128], F16, tag="attT")
                            for kc in range(2):
                                pt = psY.tile([128, 128], F32, tag="sm")
                                nc.tensor.transpose(
                                    pt[:], att[:, kc * 128:(kc + 1) * 128],
                                    ident32[:])
                                nc.vector.tensor_copy(attT[:, kc, :], pt[:])
                            for kc in range(2):
                                nc.tensor.matmul(
                                    pso[off:off + 64, :],
                                    V[:, kc, dvc * 128 + off:dvc * 128 + off + 64],
                                    attT[:, kc, :],
                                    start=(kc == 0), stop=(kc == 1))
                        nc.vector.tensor_copy(oT[:, dvc, tg * 128:(tg + 1) * 128],
                                              pso[:])

                # AO projection + residual
                b_ao = bpool.tile([1, D], F16, tag="bao")
                dma(b_ao[:], t_baor[i, :, :])
                if even:
                    attnH = apool.tile([128, 2, D], F16, tag="n1")
                psa = [[psX.tile([128, 384], F32, tag="acc", name=f"ao{tg}{n}")
                        for n in range(2)] for tg in range(2)]
                for tg in range(2):
                    for n in range(2):
                        sl = slice(n * 384, (n + 1) * 384)
                        nc.tensor.matmul(psa[tg][n][:], onesc[:1, :128],
                                         b_ao[:, sl], start=True, stop=False)
                for c in range(KC):
                    w_ch = wpool.tile([128, D], F16, tag="wchunk", name="wao_c")
                    dma(w_ch[:], t_Wao[i, c, :, :])
                    for tg in range(2):
                        for n in range(2):
                            sl = slice(n * 384, (n + 1) * 384)
                            nc.tensor.matmul(psa[tg][n][:],
                                             oT[:, c, tg * 128:(tg + 1) * 128],
                                             w_ch[:, sl],
                                             start=False, stop=(c == KC - 1))
                for tg in range(2):
                    for n in range(2):
                        sl = slice(n * 384, (n + 1) * 384)
                        if even:
                            nc.vector.tensor_copy(attnH[:, tg, sl], psa[tg][n][:])
                        nc.vector.tensor_tensor(out=res1[:, tg, sl],
                                                in0=psa[tg][n][:],
                                                in1=h[:, tg, sl], op=AluOp.add)

                # LN2 -> n2 -> n2T
                n2 = apool.tile([128, 2, D], F32, tag="n2")
                layernorm_16(res1, n2, eps12)
                n2T = apool.tile([128, KC, 256], F16, tag="n2T")
                transpose6(n2, n2T)

                # FFN up (transposed out) + gelu
                b_i = bpool.tile([128, HC], F32, tag="bi")
                dma(b_i[:], t_bi[i, :, :])
                interT = apool.tile([128, HC, 256], F16, tag="bigact")
                for g in range(6):
                    pss = [psX.tile([128, 256], F32, tag="acc", name=f"up{g}{j}")
                           for j in range(4)]
                    for c in range(KC):
                        w_ch = wpool.tile([128, 4 * 128], F16, tag="wchunk",
                                          name="wi_c")
                        dma(w_ch[:], t_Wi[i, g, c, :, :])
                        for j in range(4):
                            nc.tensor.matmul(pss[j][:],
                                             w_ch[:, j * 128:(j + 1) * 128],
                                             n2T[:, c, :],
                                             start=(c == 0), stop=(c == KC - 1))
                    for j in range(4):
                        hc = g * 4 + j
                        act_gelu(interT[:, hc, :], pss[j][:],
                                 bias=b_i[:, hc:hc + 1])

                # FFN down
                b_o = bpool.tile([1, D], F16, tag="bo")
                dma(b_o[:], t_bor[i, :, :])
                if even:
                    base = apool.tile([128, 2, D], F32, tag="base")
                psd2 = [[psX.tile([128, 384], F32, tag="acc", name=f"dn{tg}{n}")
                         for n in range(2)] for tg in range(2)]
                for tg in range(2):
                    for n in range(2):
                        sl = slice(n * 384, (n + 1) * 384)
                        nc.tensor.matmul(psd2[tg][n][:], onesc[:1, :128],
                                         b_o[:, sl], start=True, stop=False)
                for c in range(HC):
                    w_ch = wpool.tile([128, D], F16, tag="wchunk", name="wo_c")
                    dma(w_ch[:], t_Wo[i, c, :, :])
                    for tg in range(2):
                        for n in range(2):
                            sl = slice(n * 384, (n + 1) * 384)
                            nc.tensor.matmul(psd2[tg][n][:],
                                             interT[:, c, tg * 128:(tg + 1) * 128],
                                             w_ch[:, sl],
                                             start=False, stop=(c == HC - 1))
                for tg in range(2):
                    for n in range(2):
                        sl = slice(n * 384, (n + 1) * 384)
                        if not even:
                            nc.vector.tensor_tensor(out=h[:, tg, sl],
                                                    in0=psd2[tg][n][:],
                                                    in1=res1[:, tg, sl],
                                                    op=AluOp.add)
                        else:
                            nc.vector.tensor_copy(base[:, tg, sl], psd2[tg][n][:])

                if not even:
                    continue

                # ================= MoE / classifier tail =================
                # base stats (mu, ms = var + mu^2), transposed to rows
                mums = mpool.tile([128, 2, 2], F32, tag="mums")   # [:, tg, (mu,ms)]
                for tg in range(2):
                    st = mpool.tile([128, 3, 6], F32, tag="lnst")
                    xs = base[:, tg, :].rearrange("p (a b) -> p a b", a=3)
                    for a in range(3):
                        nc.vector.bn_stats(st[:, a, :], xs[:, a, :])
                    mv = mpool.tile([128, 2], F32, tag="lnmv")
                    nc.vector.bn_aggr(mv[:], st[:])
                    nc.vector.tensor_copy(mums[:, tg, 0:1], mv[:, 0:1])
                    # ms = var + mu^2
                    musq = mpool.tile([128, 1], F32, tag="musq")
                    nc.vector.tensor_tensor(out=musq[:], in0=mv[:, 0:1],
                                            in1=mv[:, 0:1], op=AluOp.mult)
                    nc.vector.tensor_tensor(out=mums[:, tg, 1:2], in0=mv[:, 1:2],
                                            in1=musq[:], op=AluOp.add)
                muT = mpool.tile([1, 256], F32, tag="muT")
                msT = mpool.tile([1, 256], F32, tag="msT")
                for tg in range(2):
                    pt = psY.tile([1, 128], F32, tag="sm")
                    nc.tensor.transpose(pt[:], mums[:, tg, 0:1], ident32[:])
                    nc.vector.tensor_copy(muT[:, tg * 128:(tg + 1) * 128], pt[:])
                    pt2 = psY.tile([1, 128], F32, tag="sm")
                    nc.tensor.transpose(pt2[:], mums[:, tg, 1:2], ident32[:])
                    nc.vector.tensor_copy(msT[:, tg * 128:(tg + 1) * 128], pt2[:])

                baseT = apool.tile([128, KC, 256], F16, tag="n1T")
                transpose6(base, baseT)

                # pooled^T [128, KC, 112] (cols ordered (d, tg*4+bl) after scatter)
                pooledT = apool.tile([128, KC, 112], F16, tag="pooledT")
                pview = pooledT.rearrange("p c (d g) -> p c d g", g=8)
                for c in range(KC):
                    for tg in range(2):
                        ps = psY.tile([128, 4 * ND], F32, tag="sm")
                        nc.tensor.matmul(ps[:], attnH[:, tg, c * 128:(c + 1) * 128],
                                         c_Mpool[:, tg, :], start=True, stop=True)
                        pv = ps[:].rearrange("p (d g) -> p d g", g=4)
                        nc.vector.tensor_copy(pview[:, c, :, tg * 4:tg * 4 + 4], pv)

                # z in disease-groups of 4 (partition = batch-slot j)
                c_clng = wmpool.tile([8, ND, D2], F16, tag="clng")
                dma(c_clng[:], t_clngB[e, :, :, :])
                c_clnb = wmpool.tile([8, ND, D2], F16, tag="clnb")
                dma(c_clnb[:], t_clnbB[e, :, :, :])
                c_cw2 = wmpool.tile([8, ND, D2], F16, tag="cw2")
                dma(c_cw2[:], t_cw2rep[e, :, :, :])
                c_cb2 = wmpool.tile([8, ND], F32, tag="cb2")
                dma(c_cb2[:], t_cb2rep[e, :, :])
                preds = mpool.tile([8, ND], F32, tag="preds")
                for dg0 in range(0, ND, 4):
                    ng = min(4, ND - dg0)
                    b_c1 = bpool.tile([1, 4, D2], F16, tag="bc1")
                    dma(b_c1[:, :ng, :],
                        t_cb1r[e, :, dg0 * D2:(dg0 + ng) * D2]
                        .rearrange("o (a b) -> o a b", a=ng))
                    zt = spool.tile([8, 4, D2], F32, tag="zt")
                    for dd in range(ng):
                        d = dg0 + dd
                        ps = psY.tile([8, D2], F32, tag="sm")
                        nc.tensor.matmul(ps[:], onesc[:1, :8], b_c1[:, dd, :],
                                         start=True, stop=False)
                        for c in range(KC):
                            w_ch = wpool.tile([128, D2], F16, tag="wchunk",
                                              name="wc1_c")
                            dma(w_ch[:], t_cW1[e, d, c, :, :])
                            nc.tensor.matmul(ps[:],
                                             pooledT[:, c, d * 8:(d + 1) * 8],
                                             w_ch[:],
                                             start=False, stop=(c == KC - 1))
                        nc.scalar.activation(zt[:, dd, :], ps[:], Act.Copy)
                    zv = zt[:, :ng, :]
                    zmean = mpool.tile([8, 4], F32, tag="zmean")
                    nc.vector.reduce_sum(zmean[:, :ng], zv,
                                         axis=mybir.AxisListType.X)
                    nc.vector.tensor_scalar(out=zmean[:, :ng], in0=zmean[:, :ng],
                                            scalar1=1.0 / D2, scalar2=None,
                                            op0=AluOp.mult)
                    zw = spool.tile([8, 4, D2], F32, tag="zw")
                    nc.vector.tensor_tensor(out=zw[:, :ng, :], in0=zv, in1=zv,
                                            op=AluOp.mult)
                    zms = mpool.tile([8, 4], F32, tag="zms")
                    nc.vector.reduce_sum(zms[:, :ng], zw[:, :ng, :],
                                         axis=mybir.AxisListType.X)
                    zvar = mpool.tile([8, 4], F32, tag="zvar")
                    nc.vector.tensor_scalar(out=zvar[:, :ng], in0=zms[:, :ng],
                                            scalar1=1.0 / D2, scalar2=None,
                                            op0=AluOp.mult)
                    zmsq = mpool.tile([8, 4], F32, tag="zmsq")
                    nc.vector.tensor_tensor(out=zmsq[:, :ng], in0=zmean[:, :ng],
                                            in1=zmean[:, :ng], op=AluOp.mult)
                    nc.vector.tensor_tensor(out=zvar[:, :ng], in0=zvar[:, :ng],
                                            in1=zmsq[:, :ng], op=AluOp.subtract)
                    zrs = mpool.tile([8, 4], F32, tag="zrs")
                    nc.scalar.activation(zrs[:, :ng], zvar[:, :ng], Act.Sqrt,
                                         bias=eps5a[:8, :], scale=1.0)
                    nc.vector.reciprocal(zrs[:, :ng], zrs[:, :ng])
                    nc.vector.tensor_tensor(
                        out=zw[:, :ng, :], in0=zv,
                        in1=zmean[:, :ng].to_broadcast((8, ng, D2)),
                        op=AluOp.subtract)
                    nc.vector.tensor_tensor(
                        out=zw[:, :ng, :], in0=zw[:, :ng, :],
                        in1=zrs[:, :ng].to_broadcast((8, ng, D2)),
                        op=AluOp.mult)
                    nc.vector.tensor_tensor(out=zw[:, :ng, :], in0=zw[:, :ng, :],
                                            in1=c_clng[:, dg0:dg0 + ng, :],
                                            op=AluOp.mult)
                    nc.vector.tensor_tensor(out=zw[:, :ng, :], in0=zw[:, :ng, :],
                                            in1=c_clnb[:, dg0:dg0 + ng, :],
                                            op=AluOp.add)
                    if not sim_gelu:
                        nc.scalar.activation(zt[:, :ng, :], zw[:, :ng, :],
                                             Act.Gelu)
                    else:
                        zu = spool.tile([8, 4, D2], F32, tag="zu")
                        nc.vector.tensor_tensor(out=zu[:, :ng, :],
                                                in0=zw[:, :ng, :],
                                                in1=zw[:, :ng, :], op=AluOp.mult)
                        nc.vector.tensor_tensor(out=zu[:, :ng, :],
                                                in0=zu[:, :ng, :],
                                                in1=zw[:, :ng, :], op=AluOp.mult)
                        nc.vector.tensor_scalar(out=zu[:, :ng, :],
                                                in0=zu[:, :ng, :],
                                                scalar1=0.044715, scalar2=None,
                                                op0=AluOp.mult)
                        nc.vector.tensor_tensor(out=zu[:, :ng, :],
                                                in0=zu[:, :ng, :],
                                                in1=zw[:, :ng, :], op=AluOp.add)
                        nc.scalar.activation(zu[:, :ng, :], zu[:, :ng, :],
                                             Act.Tanh, scale=0.7978845608)
                        nc.vector.tensor_scalar(out=zu[:, :ng, :],
                                                in0=zu[:, :ng, :], scalar1=1.0,
                                                scalar2=0.5, op0=AluOp.add,
                                                op1=AluOp.mult)
                        nc.vector.tensor_tensor(out=zt[:, :ng, :],
                                                in0=zu[:, :ng, :],
                                                in1=zw[:, :ng, :], op=AluOp.mult)
                    nc.vector.tensor_tensor(out=zt[:, :ng, :], in0=zt[:, :ng, :],
                                            in1=c_cw2[:, dg0:dg0 + ng, :],
                                            op=AluOp.mult)
                    nc.vector.reduce_sum(preds[:, dg0:dg0 + ng], zt[:, :ng, :],
                                         axis=mybir.AxisListType.X)
                nc.vector.tensor_tensor(out=preds[:], in0=preds[:], in1=c_cb2[:],
                                        op=AluOp.add)
                da = mpool.tile([8, ND], F16, tag="da")
                nc.vector.tensor_scalar(out=da[:], in0=preds[:], scalar1=0.0,
                                        scalar2=None, op0=AluOp.is_gt)

                # routing weights w [128, tg, E] f32
                w_rt = spool.tile([128, 2, E], F32, tag="wrt")
                nact = mpool.tile([128, 2], F32, tag="nact")
                for tg in range(2):
                    psd = psY.tile([128, ND], F32, tag="sm")
                    nc.tensor.matmul(psd[:], c_Ind2[:, tg, :], da[:],
                                     start=True, stop=True)
                    nc.vector.tensor_tensor(out=w_rt[:, tg, 0:ND], in0=psd[:],
                                            in1=c_maskS[:], op=AluOp.mult)
                    nc.vector.reduce_sum(nact[:, tg:tg + 1], w_rt[:, tg, 0:ND],
                                         axis=mybir.AxisListType.X)
                    nc.vector.tensor_scalar(out=nact[:, tg:tg + 1],
                                            in0=nact[:, tg:tg + 1],
                                            scalar1=1.0, scalar2=None,
                                            op0=AluOp.add)
                rnact = mpool.tile([128, 2], F32, tag="rnact")
                nc.vector.reciprocal(rnact[:], nact[:])
                for tg in range(2):
                    nc.vector.tensor_scalar(out=w_rt[:, tg, 0:ND],
                                            in0=w_rt[:, tg, 0:ND],
                                            scalar1=rnact[:, tg:tg + 1],
                                            scalar2=None, op0=AluOp.mult)
                    nc.vector.tensor_copy(w_rt[:, tg, ND:E], rnact[:, tg:tg + 1])
                wT = mpool.tile([E, 256], F32, tag="wT")
                for tg in range(2):
                    pt = psY.tile([E, 128], F32, tag="sm")
                    nc.tensor.transpose(pt[:], w_rt[:, tg, :], ident32[:])
                    nc.vector.tensor_copy(wT[:, tg * 128:(tg + 1) * 128], pt[:])

                # LoRA rails
                w_au = wmpool.tile([128, KC, ER], F16, tag="wau")
                dma(w_au[:], t_AuT[e, :, :, :])
                b_lup = wmpool.tile([1, ER], F16, tag="blup")
                dma(b_lup[:], t_lupbr[e, :, :])
                w_ad = wmpool.tile([128, HC, ER], F16, tag="wad")
                dma(w_ad[:], t_AdT[e, :, :, :])
                w_cb = wmpool.tile([ER, ER], F16, tag="wcb")
                dma(w_cb[:], t_Cblk[e, :, :])
                ps = psY.tile([ER, 256], F32, tag="sm")
                nc.tensor.matmul(ps[:], b_lup[:, :], onesc[:1, :256],
                                 start=True, stop=False)
                for c in range(KC):
                    nc.tensor.matmul(ps[:], w_au[:, c, :], n2T[:, c, :],
                                     start=False, stop=(c == KC - 1))
                lup_rT = spool.tile([ER, 256], F16, tag="luprT")
                nc.vector.tensor_copy(lup_rT[:], ps[:])

                ps2 = psY.tile([ER, 256], F32, tag="sm")
                nc.tensor.matmul(ps2[:], w_cb[:], lup_rT[:], start=True, stop=False)
                for c in range(HC):
                    nc.tensor.matmul(ps2[:], w_ad[:, c, :], interT[:, c, :],
                                     start=False, stop=(c == HC - 1))
                ldr16 = spool.tile([ER, 256], F16, tag="ldr16")
                nc.vector.tensor_copy(ldr16[:], ps2[:])
                ldr32 = spool.tile([ER, 256], F32, tag="ldr32")
                nc.vector.tensor_copy(ldr32[:], ps2[:])

                # G^T (cross term, x2 folded in BdfD) and quad term
                w_bdd = wmpool.tile([128, KC, ER], F16, tag="wbdd")
                dma(w_bdd[:], t_BdfD[e, :, :, :])
                psg = psY.tile([ER, 256], F32, tag="sm")
                for c in range(KC):
                    nc.tensor.matmul(psg[:], w_bdd[:, c, :], baseT[:, c, :],
                                     start=(c == 0), stop=(c == KC - 1))
                Pcross = spool.tile([ER, 256], F16, tag="pcross")
                nc.vector.tensor_tensor(out=Pcross[:], in0=psg[:],
                                        in1=ldr32[:], op=AluOp.mult)

                w_gram = wmpool.tile([ER, ER], F16, tag="wgram")
                dma(w_gram[:], t_GramB[e, :, :])
                psq = psY.tile([ER, 256], F32, tag="sm")
                nc.tensor.matmul(psq[:], w_gram[:], ldr16[:],
                                 start=True, stop=True)
                Pquad = spool.tile([ER, 256], F16, tag="pquad")
                nc.vector.tensor_tensor(out=Pquad[:], in0=psq[:], in1=ldr32[:],
                                        op=AluOp.mult)

                # mu_e^T [E, 256]
                w_bdm = wmpool.tile([ER, E], F16, tag="wbdm")
                dma(w_bdm[:], t_BdmSeg1[e, :, :])
                muT16 = mpool.tile([1, 256], F16, tag="muT16")
                nc.vector.tensor_copy(muT16[:], muT[:])
                msT16 = mpool.tile([1, 256], F16, tag="msT16")
                nc.vector.tensor_copy(msT16[:], msT[:])
                psmu = psY.tile([E, 256], F32, tag="sm")
                nc.tensor.matmul(psmu[:], w_bdm[:], ldr16[:], start=True, stop=False)
                nc.tensor.matmul(psmu[:], onesc[:1, :E], muT16[:],
                                 start=False, stop=True)
                muE = mpool.tile([E, 256], F32, tag="muE")
                nc.vector.tensor_copy(muE[:], psmu[:])

                # ms^T then var, rho
                psms = psY.tile([E, 256], F32, tag="sm")
                nc.tensor.matmul(psms[:], c_SegSel0[:], Pcross[:],
                                 start=True, stop=False)
                nc.tensor.matmul(psms[:], c_SegSel0[:], Pquad[:],
                                 start=False, stop=False)
                nc.tensor.matmul(psms[:], onesc[:1, :E], msT16[:],
                                 start=False, stop=True)
                musqE = mpool.tile([E, 256], F32, tag="musqE")
                nc.vector.tensor_tensor(out=musqE[:], in0=muE[:], in1=muE[:],
                                        op=AluOp.mult)
                varE = mpool.tile([E, 256], F32, tag="varE")
                nc.vector.tensor_tensor(out=varE[:], in0=psms[:], in1=musqE[:],
                                        op=AluOp.subtract)
                sqE = mpool.tile([E, 256], F32, tag="sqE")
                nc.scalar.activation(sqE[:], varE[:], Act.Sqrt,
                                     bias=eps5a[:E, :], scale=1.0)
                rho = mpool.tile([E, 256], F32, tag="rho")
                nc.vector.reciprocal(rho[:], sqE[:])

                # s_e = w * rho ; pack [sE | sE*muE] -> column sums -> scal/off
                packSO = mpool.tile([E, 512], F16, tag="packSO")
                nc.vector.tensor_tensor(out=packSO[:, 0:256], in0=wT[:], in1=rho[:],
                                        op=AluOp.mult)
                nc.vector.tensor_tensor(out=packSO[:, 256:512],
                                        in0=packSO[:, 0:256], in1=muE[:],
                                        op=AluOp.mult)
                psso = psY.tile([1, 512], F32, tag="sm")
                nc.tensor.matmul(psso[:], ones15[:], packSO[:],
                                 start=True, stop=True)
                soT = mpool.tile([1, 512], F32, tag="soT")
                nc.vector.tensor_copy(soT[:], psso[:])
                scal = mpool.tile([128, 2], F32, tag="scal")
                off = mpool.tile([128, 2], F32, tag="off")
                for tg in range(2):
                    pt = psY.tile([128, 1], F32, tag="sm")
                    nc.tensor.transpose(pt[:], soT[:, tg * 128:(tg + 1) * 128],
                                        ident32[:1, :1])
                    nc.vector.tensor_copy(scal[:, tg:tg + 1], pt[:])
                    pt2 = psY.tile([128, 1], F32, tag="sm")
                    nc.tensor.transpose(pt2[:],
                                        soT[:, 256 + tg * 128:256 + (tg + 1) * 128],
                                        ident32[:1, :1])
                    nc.vector.tensor_copy(off[:, tg:tg + 1], pt2[:])

                # ls^T = ldown_r^T * repeat(s_e)
                psrep = psY.tile([ER, 256], F32, tag="sm")
                nc.tensor.matmul(psrep[:], c_RepSel[:], packSO[:, 0:256],
                                 start=True, stop=True)
                srep = mpool.tile([ER, 256], F32, tag="srep")
                nc.vector.tensor_copy(srep[:], psrep[:])
                lsT = spool.tile([ER, 256], F16, tag="lsT")
                nc.vector.tensor_tensor(out=lsT[:], in0=srep[:], in1=ldr32[:],
                                        op=AluOp.mult)

                # final: h = (res1 - off) + (base*scal + ldown_mix)
                w_bdf = wmpool.tile([ER, D], F16, tag="wbdf")
                dma(w_bdf[:], t_Bdfr[e, :, :])
                for tg in range(2):
                    for n in range(2):
                        sl = slice(n * 384, (n + 1) * 384)
                        psf = psX.tile([128, 384], F32, tag="acc", name="psf")
                        nc.tensor.matmul(psf[:],
                                         lsT[:, tg * 128:(tg + 1) * 128],
                                         w_bdf[:, sl], start=True, stop=True)
                        tmp = spool.tile([128, 384], F32, tag="ffn_tmp")
                        nc.vector.scalar_tensor_tensor(
                            out=tmp[:], in0=base[:, tg, sl],
                            scalar=scal[:, tg:tg + 1],
                            in1=psf[:], op0=AluOp.mult, op1=AluOp.add)
                        nc.vector.scalar_tensor_tensor(
                            out=h[:, tg, sl], in0=res1[:, tg, sl],
                            scalar=off[:, tg:tg + 1], in1=tmp[:],
                            op0=AluOp.subtract, op1=AluOp.add)

            # ---------------- final LN ----------------
            hf = apool.tile([128, 2, D], F32, tag="base")
            layernorm_16(h, hf, eps12)   # writes f32 since tile dtype f32
            for tg in range(2):
                ot = spool.tile([128, D], F32, tag="z")
                nc.vector.tensor_tensor(out=ot[:], in0=hf[:, tg, :],
                                        in1=c_fgB[:], op=AluOp.mult)
                nc.vector.tensor_tensor(out=ot[:], in0=ot[:], in1=c_fbB[:],
                                        op=AluOp.add)
                dma(t_out[tg, :, :], ot[:])

    nc.compile()
    return nc


_CACHE = {}


def _get_nc(sim_gelu=False):
    key = ("nc", sim_gelu)
    if key not in _CACHE:
        _CACHE[key] = _build(sim_gelu)
    return _CACHE[key]


def kernel(**inputs):
    inputs = {k: np.asarray(v) for k, v in inputs.items()}
    P = _prep(inputs)
    shards = _shard_x0(inputs)
    nc = _get_nc()
    base_map = {k: np.ascontiguousarray(v) for k, v in P.items()}
    in_maps = []
    for c in range(NC):
        m = dict(base_map)
        m["x0"] = np.ascontiguousarray(shards[c])
        in_maps.append(m)
    res = bass_utils.run_bass_kernel_spmd(nc, in_maps, core_ids=list(range(NC)))
    out = np.zeros((B, S, D), f32)
    for c in range(NC):
        oc = res.results[c]["out"].reshape(NT, D)
        for bl in range(BPC):
            out[c * BPC + bl] = oc[bl * TS: bl * TS + S]
    return out

